# revision 1
# baseline (speedup 1.0000x reference)
"""Trainium2 Bass kernel for nn_DistanceKMeanLoss (mean k-NN distance).

Data-parallel over batch B=16 across 8 NeuronCores (2 batches/core), with
host-built spatial candidate pruning:

Host (numpy, per batch): Morton-order the N=4096 points.  For every 32-query
sub-block, build a candidate set provably containing each query's (k+1)
nearest neighbors: a grid box-count gives a conservative per-point radius
upper bound, the resulting conservative set is refined to the exact union of
per-query balls of radius (18th-smallest in-set distance).  Four adjacent
sub-blocks form a 128-query "super-block"; its column set is the union of
the four candidate sets (own 128 queries first, so query i's self column is
column i).  Mean union width is ~190 columns instead of 4096 — any point
outside a row's candidate ball is provably farther than its k-th neighbor,
so top-k over the super-block union is exact.

Device (per super-block): one K=5 fp32 GEMM (augmented factors:
s = -d2 = 2q.c - |q|^2 - |c|^2) into PSUM; scalar engine copies PSUM->SBUF;
gpsimd adds a -1e30 diagonal to knock out self columns; the vector engine
extracts each row's top-k largest s values (= k smallest distances) with
max8/match_replace passes; after clamping to <= 0, one scalar-engine Sqrt
activation with fused accumulation emits each row's sum of k NN distances.
Host sums all rows / (B*N*k).
"""

import sys

sys.path.insert(0, "/opt/trn_rl_repo")

import numpy as np

import concourse.bacc as bacc
import concourse.tile as tile
import concourse.mybir as mybir
from concourse.bass_utils import run_bass_kernel_spmd

B, N, D = 16, 4096, 3
N_CORES = 8
BATCH_PER_CORE = B // N_CORES
SUB = 32
NSUB = N // SUB
NSUPER = BATCH_PER_CORE * (N // 128)   # 64 supers per core
NEG_BIG = -1e30
DUMMY = 100.0

_compiled_cache = {}


def _morton3(q):
    out = np.zeros(len(q), dtype=np.uint64)
    for b in range(10):
        for d in range(3):
            out |= ((q[:, d].astype(np.uint64) >> b) & 1) << np.uint64(3 * b + d)
    return out


def _build_batch_index(P, kneed, h=0.35):
    """Morton order + per-128-query-super candidate index lists (into the
    morton-ordered points), own 128 queries first."""
    n = len(P)
    lo, hi = P.min(0) - 1e-4, P.max(0) + 1e-4
    G = np.maximum(((hi - lo) / h).astype(int) + 1, 1)
    ci = np.minimum(((P - lo) / h).astype(int), G - 1)
    H = np.zeros(tuple(G + 1), dtype=np.int32)
    np.add.at(H, (ci[:, 0] + 1, ci[:, 1] + 1, ci[:, 2] + 1), 1)
    H = H.cumsum(0).cumsum(1).cumsum(2)

    def boxcount(c, w):
        l0 = np.clip(c[:, 0] - w, 0, G[0]); u0 = np.clip(c[:, 0] + w + 1, 0, G[0])
        l1 = np.clip(c[:, 1] - w, 0, G[1]); u1 = np.clip(c[:, 1] + w + 1, 0, G[1])
        l2 = np.clip(c[:, 2] - w, 0, G[2]); u2 = np.clip(c[:, 2] + w + 1, 0, G[2])
        return (H[u0, u1, u2] - H[l0, u1, u2] - H[u0, l1, u2] - H[u0, u1, l2]
                + H[l0, l1, u2] + H[l0, u1, l2] + H[u0, l1, l2] - H[l0, l1, l2])

    wq = np.full(n, 64, dtype=int)
    unresolved = np.ones(n, dtype=bool)
    for w in range(1, 64):
        idx = np.where(unresolved)[0]
        if not len(idx):
            break
        done = boxcount(ci[idx], w) >= kneed
        wq[idx[done]] = w
        unresolved[idx[done]] = False
    Rbox = np.sqrt(3.0) * (wq + 1) * h

    q = np.minimum(((P - lo) / max((hi - lo).max(), 1e-9) * 1023).astype(int),
                   1023)
    order = np.argsort(_morton3(q), kind="stable")
    Ps = P[order]
    Rs = Rbox[order]

    super_lists = []
    for S in range(n // 128):
        keep = np.zeros(n, dtype=bool)
        for s in range(4 * S, 4 * S + 4):
            blkP = Ps[s * SUB:(s + 1) * SUB]
            lo_b, hi_b = blkP.min(0), blkP.max(0)
            d_aabb = np.linalg.norm(Ps - np.clip(Ps, lo_b, hi_b), axis=1)
            Rblk = Rs[s * SUB:(s + 1) * SUB].max()
            cands = np.where(d_aabb <= Rblk)[0]
            if len(cands) > kneed:
                d2 = ((blkP[:, None, :].astype(np.float64)
                       - Ps[cands][None, :, :].astype(np.float64)) ** 2).sum(-1)
                kk = min(kneed - 1, d2.shape[1] - 1)
                kth = np.partition(d2, kk, axis=1)[:, kk]
                sel = (d2 <= kth[:, None] * (1 + 1e-4) + 1e-5).any(axis=0)
                keep[cands[sel]] = True
            else:
                keep[cands] = True
        keep[S * 128:(S + 1) * 128] = False   # own queries prepended below
        others = np.where(keep)[0]
        idx = np.concatenate([np.arange(S * 128, (S + 1) * 128), others])
        super_lists.append(idx)
    return order, Ps, super_lists


def _split16(v):
    hi = v.astype(np.float16)
    lo = (v - hi.astype(np.float32)).astype(np.float16)
    return hi, lo


def _lhsT_cols(pts, s):
    """fp16 hi/lo augmented query factors, K=13 (see _rhs_cols)."""
    phi, plo = _split16(pts)
    shi, slo = _split16(s)
    out = np.empty((13, len(pts)), dtype=np.float16)
    out[0:3] = (2.0 * phi.astype(np.float32)).astype(np.float16).T
    out[3:6] = (2.0 * plo.astype(np.float32)).astype(np.float16).T
    out[6:9] = out[0:3]
    out[9] = -shi
    out[10] = -slo
    out[11] = -1.0
    out[12] = -1.0
    return out


def _rhs_cols(pts, s):
    """fp16 hi/lo augmented candidate factors:
    dot = 2q_hi.c_hi + 2q_lo.c_hi + 2q_hi.c_lo - s_q - s_c = -d2."""
    phi, plo = _split16(pts)
    shi, slo = _split16(s)
    out = np.empty((13, len(pts)), dtype=np.float16)
    out[0:3] = phi.T
    out[3:6] = phi.T
    out[6:9] = plo.T
    out[9] = 1.0
    out[10] = 1.0
    out[11] = shi
    out[12] = slo
    return out


def build_inputs(pcs, k):
    """Per-core input maps + the common per-super width list."""
    kneed = k + 2
    sq = np.sum(pcs.astype(np.float64) ** 2, axis=-1).astype(np.float32)

    core_supers = [[] for _ in range(N_CORES)]   # (Ps, s_m, idx)
    for c in range(N_CORES):
        for bl in range(BATCH_PER_CORE):
            b = c * BATCH_PER_CORE + bl
            order, Ps, super_lists = _build_batch_index(pcs[b], kneed)
            s_m = sq[b][order]
            for S in range(N // 128):
                core_supers[c].append((Ps, s_m, super_lists[S]))

    # exact scan width (cross-core max); layout offsets padded to 16 cols
    W_super = []
    for si in range(NSUPER):
        w = max(len(core_supers[c][si][2]) for c in range(N_CORES))
        W_super.append(max(w, 144))
    W_pad = [((w + 15) // 16) * 16 for w in W_super]
    offs = np.concatenate([[0], np.cumsum(W_pad)]).astype(int)
    total = int(offs[-1])

    dummy_pts = np.full((1, 3), DUMMY, dtype=np.float32)
    dummy_col = _rhs_cols(dummy_pts,
                          np.array([3 * DUMMY * DUMMY], dtype=np.float32))
    diagm = np.eye(128, dtype=np.float32) * np.float32(NEG_BIG)

    in_maps = []
    for c in range(N_CORES):
        RC = np.empty((13, total), dtype=np.float16)
        LQ = np.empty((13, NSUPER * 128), dtype=np.float16)
        for si in range(NSUPER):
            Ps, s_m, idx = core_supers[c][si]
            base = int(offs[si])
            wp = int(offs[si + 1]) - base
            cols = _rhs_cols(Ps[idx], s_m[idx])
            RC[:, base:base + len(idx)] = cols
            RC[:, base + len(idx):base + wp] = dummy_col
            LQ[:, si * 128:(si + 1) * 128] = _lhsT_cols(Ps[idx[:128]],
                                                        s_m[idx[:128]])
        in_maps.append({"RC": RC, "LQ": LQ, "diagm": diagm})
    return in_maps, W_super, total


def _build_kernel(k, W_super, total):
    n_rounds = (k + 7) // 8
    n_slots = n_rounds * 8
    max_w = max(W_super)

    nc = bacc.Bacc("TRN2", target_bir_lowering=False, debug=False,
                   num_devices=N_CORES)
    RC_ext = nc.dram_tensor("RC", [13, total], mybir.dt.float16,
                            kind="ExternalInput").ap()
    LQ_ext = nc.dram_tensor("LQ", [13, NSUPER * 128], mybir.dt.float16,
                            kind="ExternalInput").ap()
    diag_ext = nc.dram_tensor("diagm", [128, 128], mybir.dt.float32,
                              kind="ExternalInput").ap()
    out_ext = nc.dram_tensor("rowsums", [128, 1], mybir.dt.float32,
                             kind="ExternalOutput").ap()

    offs = [0]
    for w in W_super:
        offs.append(offs[-1] + ((w + 15) // 16) * 16)

    with tile.TileContext(nc) as tc:
        with (
            tc.tile_pool(name="const", bufs=1) as const_pool,
            tc.tile_pool(name="s32", bufs=3) as s32_pool,
            tc.tile_pool(name="small", bufs=2) as small_pool,
            tc.tile_pool(name="psum", bufs=8, space="PSUM") as psum_pool,
        ):
            RC_sb = const_pool.tile([13, total], mybir.dt.float16, tag="RC")
            LQ_sb = const_pool.tile([13, NSUPER * 128], mybir.dt.float16,
                                    tag="LQ")
            diag_sb = const_pool.tile([128, 128], mybir.dt.float32, tag="diag")
            M_all = const_pool.tile([128, NSUPER * n_slots], mybir.dt.float32,
                                    tag="mall")
            nc.sync.dma_start(RC_sb[:], RC_ext[:])
            nc.sync.dma_start(LQ_sb[:], LQ_ext[:])
            nc.sync.dma_start(diag_sb[:], diag_ext[:])

            for si in range(NSUPER):
                w = W_super[si]
                s32 = s32_pool.tile([128, max_w], mybir.dt.float32, tag="sa")
                for m0 in range(0, w, 512):
                    mw = min(512, w - m0)
                    ps = psum_pool.tile([128, 512], mybir.dt.float32, tag="ps")
                    nc.tensor.matmul(
                        ps[:, :mw],
                        LQ_sb[:, si * 128:(si + 1) * 128],
                        RC_sb[:, offs[si] + m0: offs[si] + m0 + mw],
                        start=True, stop=True,
                    )
                    nc.scalar.copy(s32[:, m0:m0 + mw], ps[:, :mw])
                # self-column knockout (query i == column i) — on gpsimd to
                # keep the vector engine free for the extraction passes
                nc.gpsimd.tensor_add(s32[:, :128], s32[:, :128], diag_sb[:])
                # top-k extraction into the shared slot buffer
                mbase = si * n_slots
                cur = s32
                for r in range(n_rounds):
                    nc.vector.max(M_all[:, mbase + r * 8: mbase + (r + 1) * 8],
                                  cur[:, :w])
                    if r + 1 < n_rounds:
                        nxt = s32_pool.tile([128, max_w], mybir.dt.float32,
                                            tag="sb")
                        nc.vector.match_replace(
                            nxt[:, :w],
                            M_all[:, mbase + r * 8: mbase + (r + 1) * 8],
                            cur[:, :w], NEG_BIG)
                        cur = nxt
            # batched epilogue: clamp all slots, zero unused, sqrt + row sum
            mm = const_pool.tile([128, NSUPER * n_slots], mybir.dt.float32,
                                 tag="mmall")
            nc.vector.tensor_scalar_min(mm[:], M_all[:], 0.0)
            if n_slots > k:
                mmv = mm[:].rearrange("p (s t) -> p s t", t=n_slots)
                nc.vector.memset(mmv[:, :, k:], 0.0)
            sq_t = small_pool.tile([128, NSUPER * n_slots], mybir.dt.float32,
                                   tag="sq")
            rowsums = small_pool.tile([128, 1], mybir.dt.float32, tag="rs")
            nc.scalar.activation(
                sq_t[:], mm[:], mybir.ActivationFunctionType.Sqrt,
                bias=0.0, scale=-1.0,
                accum_out=rowsums[:],
            )
            nc.sync.dma_start(out_ext[:], rowsums[:])

    nc.compile()
    return nc


def prepare(pcs: np.ndarray, k: int):
    pcs = np.asarray(pcs, dtype=np.float32)
    in_maps, W_super, total = build_inputs(pcs, k)
    key = (k, tuple(W_super))
    if key not in _compiled_cache:
        _compiled_cache[key] = _build_kernel(k, W_super, total)
    return _compiled_cache[key], in_maps


def reduce_results(results, k: int) -> np.ndarray:
    total = 0.0
    for c in range(N_CORES):
        total += results[c]["rowsums"].astype(np.float64).sum()
    return np.float32(total / (B * N * k))


def kernel(pcs: np.ndarray, k) -> np.ndarray:
    k = int(k)
    if k <= 0:
        return np.float32(np.nan)
    nc, in_maps = prepare(pcs, k)
    res = run_bass_kernel_spmd(nc, in_maps, list(range(N_CORES)))
    return reduce_results(res.results, k)



# revision 4
# speedup vs baseline: 1.0781x; 1.0781x over previous
"""Trainium2 Bass kernel for nn_DistanceKMeanLoss (mean k-NN distance).

Data-parallel over batch B=16 across 8 NeuronCores (2 batches/core), with
host-built spatial candidate pruning:

Host (numpy, per batch): Morton-order the N=4096 points.  For every 32-query
sub-block, build a candidate set provably containing each query's (k+1)
nearest neighbors: a grid box-count gives a conservative per-point radius
upper bound, the resulting conservative set is refined to the exact union of
per-query balls of radius (18th-smallest in-set distance).  Four adjacent
sub-blocks form a 128-query "super-block"; its column set is the union of
the four candidate sets (own 128 queries first, so query i's self column is
column i).  Mean union width is ~220 columns instead of 4096 — any point
outside a row's candidate ball is provably farther than its k-th neighbor,
so top-k over the super-block union is exact.

Device layout: the 64 supers are width-sorted into 8 slots x 8 partition
groups.  Group g owns SBUF partitions [16g, 16g+13); slot j spans a fixed
column range shared by all groups, holding each group's [13,128] query
factors followed by its [13, SW_j] candidate factors.  One [128, Y] fp16
tensor therefore carries all GEMM inputs and is DMA'd in 4 column chunks
across the full 128-partition width (fast), alternating the SP/Activation
DMA queues; compute on slot j only waits for chunk j//2.

Device (per super): one K=13 fp16 GEMM (augmented hi/lo split factors:
s = -d2 = 2q.c - |q|^2 - |c|^2) into PSUM; the scalar engine downcasts
PSUM->SBUF to fp16; gpsimd adds a -60000 diagonal to knock out self
columns; the vector engine extracts each row's top-k largest s values
(= k smallest distances) with fp16 max8/match_replace passes (2x DVE
mode); after clamping to <= 0, one scalar-engine Sqrt activation with
fused accumulation emits each row's sum of k NN distances.
Host sums all rows / (B*N*k).
"""

import sys

sys.path.insert(0, "/opt/trn_rl_repo")

import numpy as np

import concourse.bacc as bacc
import concourse.tile as tile
import concourse.mybir as mybir
from concourse.bass_utils import run_bass_kernel_spmd

B, N, D = 16, 4096, 3
N_CORES = 8
BATCH_PER_CORE = B // N_CORES
SUB = 32
NSUPER = BATCH_PER_CORE * (N // 128)   # 64 supers per core
NGROUP = 4                             # partition groups (stride 32: PE quadrant bases)
NSLOT = NSUPER // NGROUP               # 16 column slots
NEG_BIG = -60000.0                     # fp16-safe knockout sentinel
DUMMY = 100.0

_compiled_cache = {}


def _morton3(q):
    out = np.zeros(len(q), dtype=np.uint64)
    for b in range(10):
        for d in range(3):
            out |= ((q[:, d].astype(np.uint64) >> b) & 1) << np.uint64(3 * b + d)
    return out


def _build_batch_index(P, kneed, h=0.35):
    """Morton order + per-128-query-super candidate index lists (into the
    morton-ordered points), own 128 queries first."""
    n = len(P)
    lo, hi = P.min(0) - 1e-4, P.max(0) + 1e-4
    G = np.maximum(((hi - lo) / h).astype(int) + 1, 1)
    ci = np.minimum(((P - lo) / h).astype(int), G - 1)
    H = np.zeros(tuple(G + 1), dtype=np.int32)
    np.add.at(H, (ci[:, 0] + 1, ci[:, 1] + 1, ci[:, 2] + 1), 1)
    H = H.cumsum(0).cumsum(1).cumsum(2)

    def boxcount(c, w):
        l0 = np.clip(c[:, 0] - w, 0, G[0]); u0 = np.clip(c[:, 0] + w + 1, 0, G[0])
        l1 = np.clip(c[:, 1] - w, 0, G[1]); u1 = np.clip(c[:, 1] + w + 1, 0, G[1])
        l2 = np.clip(c[:, 2] - w, 0, G[2]); u2 = np.clip(c[:, 2] + w + 1, 0, G[2])
        return (H[u0, u1, u2] - H[l0, u1, u2] - H[u0, l1, u2] - H[u0, u1, l2]
                + H[l0, l1, u2] + H[l0, u1, l2] + H[u0, l1, l2])

    wq = np.full(n, 64, dtype=int)
    unresolved = np.ones(n, dtype=bool)
    for w in range(1, 64):
        idx = np.where(unresolved)[0]
        if not len(idx):
            break
        done = boxcount(ci[idx], w) >= kneed
        wq[idx[done]] = w
        unresolved[idx[done]] = False
    Rbox = np.sqrt(3.0) * (wq + 1) * h

    q = np.minimum(((P - lo) / max((hi - lo).max(), 1e-9) * 1023).astype(int),
                   1023)
    order = np.argsort(_morton3(q), kind="stable")
    Ps = P[order]
    Rs = Rbox[order]

    super_lists = []
    for S in range(n // 128):
        keep = np.zeros(n, dtype=bool)
        for s in range(4 * S, 4 * S + 4):
            blkP = Ps[s * SUB:(s + 1) * SUB]
            lo_b, hi_b = blkP.min(0), blkP.max(0)
            d_aabb = np.linalg.norm(Ps - np.clip(Ps, lo_b, hi_b), axis=1)
            Rblk = Rs[s * SUB:(s + 1) * SUB].max()
            cands = np.where(d_aabb <= Rblk)[0]
            if len(cands) > kneed:
                d2 = ((blkP[:, None, :].astype(np.float64)
                       - Ps[cands][None, :, :].astype(np.float64)) ** 2).sum(-1)
                kk = min(kneed - 1, d2.shape[1] - 1)
                kth = np.partition(d2, kk, axis=1)[:, kk]
                sel = (d2 <= kth[:, None] * (1 + 1e-4) + 1e-5).any(axis=0)
                keep[cands[sel]] = True
            else:
                keep[cands] = True
        keep[S * 128:(S + 1) * 128] = False   # own queries prepended below
        others = np.where(keep)[0]
        idx = np.concatenate([np.arange(S * 128, (S + 1) * 128), others])
        super_lists.append(idx)
    return order, Ps, super_lists


def _split16(v):
    hi = v.astype(np.float16)
    lo = (v - hi.astype(np.float32)).astype(np.float16)
    return hi, lo


def _lhsT_cols(pts, s):
    """fp16 hi/lo augmented query factors, K=13 (see _rhs_cols)."""
    phi, plo = _split16(pts)
    shi, slo = _split16(s)
    out = np.empty((13, len(pts)), dtype=np.float16)
    out[0:3] = (2.0 * phi.astype(np.float32)).astype(np.float16).T
    out[3:6] = (2.0 * plo.astype(np.float32)).astype(np.float16).T
    out[6:9] = out[0:3]
    out[9] = -shi
    out[10] = -slo
    out[11] = -1.0
    out[12] = -1.0
    return out


def _rhs_cols(pts, s):
    """fp16 hi/lo augmented candidate factors:
    dot = 2q_hi.c_hi + 2q_lo.c_hi + 2q_hi.c_lo - s_q - s_c = -d2."""
    phi, plo = _split16(pts)
    shi, slo = _split16(s)
    out = np.empty((13, len(pts)), dtype=np.float16)
    out[0:3] = phi.T
    out[3:6] = phi.T
    out[6:9] = plo.T
    out[9] = 1.0
    out[10] = 1.0
    out[11] = shi
    out[12] = slo
    return out


def _layout(W_super):
    """Width-sorted slot layout shared by all cores."""
    order = np.argsort(np.asarray(W_super), kind="stable")   # ascending
    SW, C = [], [0]
    for j in range(NSLOT):
        ids = order[NGROUP * j: NGROUP * (j + 1)]
        w = max(int(W_super[s]) for s in ids)
        SW.append(((w + 15) // 16) * 16)
        C.append(C[-1] + 128 + SW[-1])
    return order, SW, C


def build_inputs(pcs, k):
    """Per-core packed [128, Y] factor maps + shared layout info."""
    kneed = k + 2
    sq = np.sum(pcs.astype(np.float64) ** 2, axis=-1).astype(np.float32)

    core_supers = [[] for _ in range(N_CORES)]   # (Ps, s_m, idx)
    for c in range(N_CORES):
        for bl in range(BATCH_PER_CORE):
            b = c * BATCH_PER_CORE + bl
            order, Ps, super_lists = _build_batch_index(pcs[b], kneed)
            s_m = sq[b][order]
            for S in range(N // 128):
                core_supers[c].append((Ps, s_m, super_lists[S]))

    # exact scan width (cross-core max)
    W_super = [max(len(core_supers[c][si][2]) for c in range(N_CORES))
               for si in range(NSUPER)]
    W_super = [max(w, 144) for w in W_super]
    sorder, SW, C = _layout(W_super)
    Y = C[-1]

    dummy_pts = np.full((1, 3), DUMMY, dtype=np.float32)
    dummy_col = _rhs_cols(dummy_pts,
                          np.array([3 * DUMMY * DUMMY], dtype=np.float32))
    diagm = np.eye(128, dtype=np.float16) * np.float16(NEG_BIG)

    in_maps = []
    for c in range(N_CORES):
        RL = np.zeros((128, Y), dtype=np.float16)
        for j in range(NSLOT):
            for g in range(NGROUP):
                sid = int(sorder[NGROUP * j + g])
                Ps, s_m, idx = core_supers[c][sid]
                p0, c0 = 32 * g, C[j]
                RL[p0:p0 + 13, c0:c0 + 128] = _lhsT_cols(Ps[idx[:128]],
                                                         s_m[idx[:128]])
                rc = _rhs_cols(Ps[idx], s_m[idx])
                RL[p0:p0 + 13, c0 + 128:c0 + 128 + len(idx)] = rc
                RL[p0:p0 + 13, c0 + 128 + len(idx):c0 + 128 + SW[j]] = dummy_col
        in_maps.append({"RL": RL, "diagm": diagm})
    return in_maps, W_super, (sorder, SW, C)


def _build_kernel(k, W_super):
    n_rounds = (k + 7) // 8
    n_slots = n_rounds * 8
    sorder, SW, C = _layout(W_super)
    Y = C[-1]
    max_w = ((max(W_super) + 15) // 16) * 16

    nc = bacc.Bacc("TRN2", target_bir_lowering=False, debug=False,
                   num_devices=N_CORES)
    RL_ext = nc.dram_tensor("RL", [128, Y], mybir.dt.float16,
                            kind="ExternalInput").ap()
    diag_ext = nc.dram_tensor("diagm", [128, 128], mybir.dt.float16,
                              kind="ExternalInput").ap()
    out_ext = nc.dram_tensor("rowsums", [128, 1], mybir.dt.float32,
                             kind="ExternalOutput").ap()

    with tile.TileContext(nc) as tc:
        with (
            tc.tile_pool(name="const", bufs=1) as const_pool,
            tc.tile_pool(name="s16", bufs=4) as s16_pool,
            tc.tile_pool(name="small", bufs=2) as small_pool,
            tc.tile_pool(name="psum", bufs=8, space="PSUM") as psum_pool,
        ):
            RL_sb = const_pool.tile([128, Y], mybir.dt.float16, tag="RL")
            diag_sb = const_pool.tile([128, 128], mybir.dt.float16, tag="diag")
            M_all = const_pool.tile([128, NSUPER * n_slots], mybir.dt.float16,
                                    tag="mall")
            nc.sync.dma_start(diag_sb[:], diag_ext[:])
            # 8 column chunks of 2 slots each, alternating DMA queues so the
            # first slots' GEMMs start after ~1/8 of the input has landed.
            for ch in range(8):
                lo, hi = C[2 * ch], C[2 * ch + 2]
                eng = nc.sync if ch % 2 == 0 else nc.scalar
                eng.dma_start(RL_sb[:, lo:hi], RL_ext[:, lo:hi])

            for j in range(NSLOT):
                for g in range(NGROUP):
                    seq = NGROUP * j + g
                    w = int(W_super[int(sorder[seq])])
                    p0, c0 = 32 * g, C[j]
                    s16 = s16_pool.tile([128, max_w], mybir.dt.float16,
                                        tag="sa")
                    for m0 in range(0, w, 512):
                        mw = min(512, w - m0)
                        ps = psum_pool.tile([128, 512], mybir.dt.float32,
                                            tag="ps")
                        nc.tensor.matmul(
                            ps[:, :mw],
                            RL_sb[p0:p0 + 13, c0:c0 + 128],
                            RL_sb[p0:p0 + 13,
                                  c0 + 128 + m0:c0 + 128 + m0 + mw],
                            start=True, stop=True,
                            tile_position=(p0, 0),
                        )
                        nc.scalar.copy(s16[:, m0:m0 + mw], ps[:, :mw])
                    # self-column knockout (query i == column i) — on gpsimd
                    # to keep the vector engine free for extraction
                    nc.gpsimd.tensor_add(s16[:, :128], s16[:, :128],
                                         diag_sb[:])
                    # top-k extraction into the shared slot buffer
                    mbase = seq * n_slots
                    cur = s16
                    for r in range(n_rounds):
                        nc.vector.max(
                            M_all[:, mbase + r * 8: mbase + (r + 1) * 8],
                            cur[:, :w])
                        if r + 1 < n_rounds:
                            nxt = s16_pool.tile([128, max_w],
                                                mybir.dt.float16, tag="sb")
                            nc.vector.match_replace(
                                nxt[:, :w],
                                M_all[:, mbase + r * 8: mbase + (r + 1) * 8],
                                cur[:, :w], NEG_BIG)
                            cur = nxt
            # batched epilogue: clamp all slots, zero unused, sqrt + row sum
            mm = const_pool.tile([128, NSUPER * n_slots], mybir.dt.float16,
                                 tag="mmall")
            nc.vector.tensor_scalar_min(mm[:], M_all[:], 0.0)
            if n_slots > k:
                mmv = mm[:].rearrange("p (s t) -> p s t", t=n_slots)
                nc.vector.memset(mmv[:, :, k:], 0.0)
            sq_t = small_pool.tile([128, NSUPER * n_slots], mybir.dt.float16,
                                   tag="sq")
            rowsums = small_pool.tile([128, 1], mybir.dt.float32, tag="rs")
            nc.scalar.activation(
                sq_t[:], mm[:], mybir.ActivationFunctionType.Sqrt,
                bias=0.0, scale=-1.0,
                accum_out=rowsums[:],
            )
            nc.sync.dma_start(out_ext[:], rowsums[:])

    nc.compile()
    return nc


def prepare(pcs: np.ndarray, k: int):
    pcs = np.asarray(pcs, dtype=np.float32)
    in_maps, W_super, _ = build_inputs(pcs, k)
    key = (k, tuple(W_super))
    if key not in _compiled_cache:
        _compiled_cache[key] = _build_kernel(k, W_super)
    return _compiled_cache[key], in_maps


def reduce_results(results, k: int) -> np.ndarray:
    total = 0.0
    for c in range(N_CORES):
        total += results[c]["rowsums"].astype(np.float64).sum()
    return np.float32(total / (B * N * k))


def kernel(pcs: np.ndarray, k) -> np.ndarray:
    k = int(k)
    if k <= 0:
        return np.float32(np.nan)
    nc, in_maps = prepare(pcs, k)
    res = run_bass_kernel_spmd(nc, in_maps, list(range(N_CORES)))
    return reduce_results(res.results, k)


# revision 7
# speedup vs baseline: 1.1930x; 1.1066x over previous
"""Trainium2 Bass kernel for nn_DistanceKMeanLoss (mean k-NN distance).

Data-parallel over batch B=16 across 8 NeuronCores (2 batches/core), with
host-built spatial candidate pruning:

Host (numpy, per batch): Morton-order the N=4096 points.  For every 32-query
sub-block, build a candidate set provably containing each query's (k+1)
nearest neighbors: a grid box-count gives a conservative per-point radius
upper bound, the resulting conservative set is refined to the exact union of
per-query balls of radius (18th-smallest in-set distance).  Four adjacent
sub-blocks form a 128-query "super-block"; its column set is the union of
the four candidate sets (own 128 queries first, so query i's self column is
column i).  Mean union width is ~220 columns instead of 4096 — any point
outside a row's candidate ball is provably farther than its k-th neighbor,
so top-k over the super-block union is exact.

Device layout: the 64 supers are width-sorted into 8 slots x 8 partition
groups.  Group g owns SBUF partitions [16g, 16g+13); slot j spans a fixed
column range shared by all groups, holding each group's [13,128] query
factors followed by its [13, SW_j] candidate factors.  One [128, Y] fp16
tensor therefore carries all GEMM inputs and is DMA'd in 4 column chunks
across the full 128-partition width (fast), alternating the SP/Activation
DMA queues; compute on slot j only waits for chunk j//2.

Device (per super): one K=13 fp16 GEMM (augmented hi/lo split factors:
s = -d2 = 2q.c - |q|^2 - |c|^2) into PSUM; the scalar engine downcasts
PSUM->SBUF to fp16; gpsimd adds a -60000 diagonal to knock out self
columns; the vector engine extracts each row's top-k largest s values
(= k smallest distances) with fp16 max8/match_replace passes (2x DVE
mode); after clamping to <= 0, one scalar-engine Sqrt activation with
fused accumulation emits each row's sum of k NN distances.
Host sums all rows / (B*N*k).
"""

import sys

sys.path.insert(0, "/opt/trn_rl_repo")

import numpy as np

import concourse.bacc as bacc
import concourse.tile as tile
import concourse.mybir as mybir
from concourse.bass_utils import run_bass_kernel_spmd

B, N, D = 16, 4096, 3
N_CORES = 8
BATCH_PER_CORE = B // N_CORES
SUB = 32
NSUPER = BATCH_PER_CORE * (N // 128)   # 64 supers per core
NGROUP = 4                             # partition groups (stride 32: PE quadrant bases)
NSLOT = NSUPER // NGROUP               # 16 column slots
NEG_BIG = -60000.0                     # fp16-safe knockout sentinel
DUMMY = 100.0

_compiled_cache = {}


def _morton3(q):
    out = np.zeros(len(q), dtype=np.uint64)
    for b in range(10):
        for d in range(3):
            out |= ((q[:, d].astype(np.uint64) >> b) & 1) << np.uint64(3 * b + d)
    return out


def _build_batch_index(P, kneed, h=0.35):
    """Morton order + per-128-query-super candidate index lists (into the
    morton-ordered points), own 128 queries first."""
    n = len(P)
    lo, hi = P.min(0) - 1e-4, P.max(0) + 1e-4
    G = np.maximum(((hi - lo) / h).astype(int) + 1, 1)
    ci = np.minimum(((P - lo) / h).astype(int), G - 1)
    H = np.zeros(tuple(G + 1), dtype=np.int32)
    np.add.at(H, (ci[:, 0] + 1, ci[:, 1] + 1, ci[:, 2] + 1), 1)
    H = H.cumsum(0).cumsum(1).cumsum(2)

    def boxcount(c, w):
        l0 = np.clip(c[:, 0] - w, 0, G[0]); u0 = np.clip(c[:, 0] + w + 1, 0, G[0])
        l1 = np.clip(c[:, 1] - w, 0, G[1]); u1 = np.clip(c[:, 1] + w + 1, 0, G[1])
        l2 = np.clip(c[:, 2] - w, 0, G[2]); u2 = np.clip(c[:, 2] + w + 1, 0, G[2])
        return (H[u0, u1, u2] - H[l0, u1, u2] - H[u0, l1, u2] - H[u0, u1, l2]
                + H[l0, l1, u2] + H[l0, u1, l2] + H[u0, l1, l2])

    wq = np.full(n, 64, dtype=int)
    unresolved = np.ones(n, dtype=bool)
    for w in range(1, 64):
        idx = np.where(unresolved)[0]
        if not len(idx):
            break
        done = boxcount(ci[idx], w) >= kneed
        wq[idx[done]] = w
        unresolved[idx[done]] = False
    Rbox = np.sqrt(3.0) * (wq + 1) * h

    q = np.minimum(((P - lo) / max((hi - lo).max(), 1e-9) * 1023).astype(int),
                   1023)
    order = np.argsort(_morton3(q), kind="stable")
    Ps = P[order]
    Rs = Rbox[order]

    super_lists = []
    for S in range(n // 128):
        keep = np.zeros(n, dtype=bool)
        for s in range(4 * S, 4 * S + 4):
            blkP = Ps[s * SUB:(s + 1) * SUB]
            lo_b, hi_b = blkP.min(0), blkP.max(0)
            d_aabb = np.linalg.norm(Ps - np.clip(Ps, lo_b, hi_b), axis=1)
            Rblk = Rs[s * SUB:(s + 1) * SUB].max()
            cands = np.where(d_aabb <= Rblk)[0]
            if len(cands) > kneed:
                d2 = ((blkP[:, None, :].astype(np.float64)
                       - Ps[cands][None, :, :].astype(np.float64)) ** 2).sum(-1)
                kk = min(kneed - 1, d2.shape[1] - 1)
                kth = np.partition(d2, kk, axis=1)[:, kk]
                sel = (d2 <= kth[:, None] * (1 + 1e-4) + 1e-5).any(axis=0)
                keep[cands[sel]] = True
            else:
                keep[cands] = True
        keep[S * 128:(S + 1) * 128] = False   # own queries prepended below
        others = np.where(keep)[0]
        idx = np.concatenate([np.arange(S * 128, (S + 1) * 128), others])
        super_lists.append(idx)
    return order, Ps, super_lists


def _split16(v):
    hi = v.astype(np.float16)
    lo = (v - hi.astype(np.float32)).astype(np.float16)
    return hi, lo


def _lhsT_cols(pts, s):
    """fp16 hi/lo augmented query factors, K=13 (see _rhs_cols)."""
    phi, plo = _split16(pts)
    shi, slo = _split16(s)
    out = np.empty((13, len(pts)), dtype=np.float16)
    out[0:3] = (2.0 * phi.astype(np.float32)).astype(np.float16).T
    out[3:6] = (2.0 * plo.astype(np.float32)).astype(np.float16).T
    out[6:9] = out[0:3]
    out[9] = -shi
    out[10] = -slo
    out[11] = -1.0
    out[12] = -1.0
    return out


def _rhs_cols(pts, s):
    """fp16 hi/lo augmented candidate factors:
    dot = 2q_hi.c_hi + 2q_lo.c_hi + 2q_hi.c_lo - s_q - s_c = -d2."""
    phi, plo = _split16(pts)
    shi, slo = _split16(s)
    out = np.empty((13, len(pts)), dtype=np.float16)
    out[0:3] = phi.T
    out[3:6] = phi.T
    out[6:9] = plo.T
    out[9] = 1.0
    out[10] = 1.0
    out[11] = shi
    out[12] = slo
    return out


def _layout(W_super):
    """Width-sorted slot layout shared by all cores."""
    order = np.argsort(np.asarray(W_super), kind="stable")   # ascending
    SW, C = [], [0]
    for j in range(NSLOT):
        ids = order[NGROUP * j: NGROUP * (j + 1)]
        w = max(int(W_super[s]) for s in ids)
        SW.append(((w + 15) // 16) * 16)
        C.append(C[-1] + 128 + SW[-1])
    return order, SW, C


def build_inputs(pcs, k):
    """Per-core packed [128, Y] factor maps + shared layout info."""
    kneed = k + 2
    sq = np.sum(pcs.astype(np.float64) ** 2, axis=-1).astype(np.float32)

    core_supers = [[] for _ in range(N_CORES)]   # (Ps, s_m, idx)
    for c in range(N_CORES):
        for bl in range(BATCH_PER_CORE):
            b = c * BATCH_PER_CORE + bl
            order, Ps, super_lists = _build_batch_index(pcs[b], kneed)
            s_m = sq[b][order]
            for S in range(N // 128):
                core_supers[c].append((Ps, s_m, super_lists[S]))

    # exact scan width (cross-core max)
    W_super = [max(len(core_supers[c][si][2]) for c in range(N_CORES))
               for si in range(NSUPER)]
    W_super = [max(w, 144) for w in W_super]
    sorder, SW, C = _layout(W_super)
    Y = C[-1]

    dummy_pts = np.full((1, 3), DUMMY, dtype=np.float32)
    dummy_col = _rhs_cols(dummy_pts,
                          np.array([3 * DUMMY * DUMMY], dtype=np.float32))
    diagm = np.eye(128, dtype=np.float16) * np.float16(NEG_BIG)

    in_maps = []
    for c in range(N_CORES):
        RL = np.zeros((128, Y), dtype=np.float16)
        for j in range(NSLOT):
            for g in range(NGROUP):
                sid = int(sorder[NGROUP * j + g])
                Ps, s_m, idx = core_supers[c][sid]
                p0, c0 = 32 * g, C[j]
                RL[p0:p0 + 13, c0:c0 + 128] = _lhsT_cols(Ps[idx[:128]],
                                                         s_m[idx[:128]])
                rc = _rhs_cols(Ps[idx], s_m[idx])
                RL[p0:p0 + 13, c0 + 128:c0 + 128 + len(idx)] = rc
                RL[p0:p0 + 13, c0 + 128 + len(idx):c0 + 128 + SW[j]] = dummy_col
        in_maps.append({"RL": RL, "diagm": diagm})
    return in_maps, W_super, (sorder, SW, C)


def _build_kernel(k, W_super):
    n_rounds = (k + 7) // 8
    n_slots = n_rounds * 8
    sorder, SW, C = _layout(W_super)
    Y = C[-1]
    max_w = ((max(W_super) + 15) // 16) * 16

    nc = bacc.Bacc("TRN2", target_bir_lowering=False, debug=False,
                   num_devices=N_CORES)
    RL_ext = nc.dram_tensor("RL", [128, Y], mybir.dt.float16,
                            kind="ExternalInput").ap()
    diag_ext = nc.dram_tensor("diagm", [128, 128], mybir.dt.float16,
                              kind="ExternalInput").ap()
    out_ext = nc.dram_tensor("rowsums", [1, NSLOT // 2], mybir.dt.float32,
                             kind="ExternalOutput").ap()

    with tile.TileContext(nc) as tc:
        with (
            tc.tile_pool(name="const", bufs=1) as const_pool,
            tc.tile_pool(name="s16", bufs=8) as s16_pool,
            tc.tile_pool(name="small", bufs=2) as small_pool,
            tc.tile_pool(name="psum", bufs=7, space="PSUM") as psum_pool,
            tc.tile_pool(name="psum1", bufs=1, space="PSUM") as psum1_pool,
        ):
            RL_sb = const_pool.tile([128, Y], mybir.dt.float16, tag="RL")
            diag_sb = const_pool.tile([128, 128], mybir.dt.float16, tag="diag")
            M_all = const_pool.tile([128, NSUPER * n_slots], mybir.dt.float16,
                                    tag="mall")
            # 8 column chunks of 2 slots each so the first slots' GEMMs
            # start after ~1/8 of the input has landed.
            nc.sync.dma_start(RL_sb[:, C[0]:C[2]], RL_ext[:, C[0]:C[2]])
            nc.sync.dma_start(diag_sb[:], diag_ext[:])
            for ch in range(1, 8):
                lo, hi = C[2 * ch], C[2 * ch + 2]
                nc.sync.dma_start(RL_sb[:, lo:hi], RL_ext[:, lo:hi])

            mm = const_pool.tile([128, NSUPER * n_slots], mybir.dt.float16,
                                 tag="mmall")
            rs = small_pool.tile([128, NSLOT // 2], mybir.dt.float32, tag="rs")
            ones = small_pool.tile([128, 1], mybir.dt.float32, tag="ones")
            nc.gpsimd.memset(ones[:], 1.0)
            for j in range(NSLOT):
                for g in range(NGROUP):
                    seq = NGROUP * j + g
                    w = int(W_super[int(sorder[seq])])
                    p0, c0 = 32 * g, C[j]
                    s16 = s16_pool.tile([128, max_w], mybir.dt.float16,
                                        tag="sa")
                    for m0 in range(0, w, 512):
                        mw = min(512, w - m0)
                        ps = psum_pool.tile([128, 512], mybir.dt.float32,
                                            tag="ps")
                        nc.tensor.matmul(
                            ps[:, :mw],
                            RL_sb[p0:p0 + 13, c0:c0 + 128],
                            RL_sb[p0:p0 + 13,
                                  c0 + 128 + m0:c0 + 128 + m0 + mw],
                            start=True, stop=True,
                            tile_position=(p0, 0),
                        )
                        nc.scalar.copy(s16[:, m0:m0 + mw], ps[:, :mw])
                    # self-column knockout (query i == column i) — on gpsimd
                    # to keep the vector engine free for extraction
                    nc.gpsimd.tensor_add(s16[:, :128], s16[:, :128],
                                         diag_sb[:])
                    # top-k extraction into the shared slot buffer
                    mbase = seq * n_slots
                    cur = s16
                    for r in range(n_rounds):
                        nc.vector.max(
                            M_all[:, mbase + r * 8: mbase + (r + 1) * 8],
                            cur[:, :w])
                        if r + 1 < n_rounds:
                            nxt = s16_pool.tile([128, max_w],
                                                mybir.dt.float16, tag="sb")
                            nc.vector.match_replace(
                                nxt[:, :w],
                                M_all[:, mbase + r * 8: mbase + (r + 1) * 8],
                                cur[:, :w], NEG_BIG)
                            cur = nxt
                # chunked epilogue on the scalar engine, hidden under the
                # vector stream: d2 = relu(-s) (clamps numeric noise), then
                # sqrt with fused row accumulation.
                if j % 2 == 1:
                    ec = j // 2
                    lo = (NGROUP * (j - 1)) * n_slots
                    hi = (NGROUP * (j + 1)) * n_slots
                    nc.scalar.activation(
                        mm[:, lo:hi], M_all[:, lo:hi],
                        mybir.ActivationFunctionType.Relu,
                        bias=0.0, scale=-1.0)
                    if n_slots > k:
                        mmv = mm[:, lo:hi].rearrange("p (s t) -> p s t",
                                                     t=n_slots)
                        nc.gpsimd.memset(mmv[:, :, k:], 0.0)
                    nc.scalar.activation(
                        mm[:, lo:hi], mm[:, lo:hi],
                        mybir.ActivationFunctionType.Sqrt,
                        bias=0.0, scale=1.0,
                        accum_out=rs[:, ec:ec + 1])
            # cross-partition reduce on PE: one scalar out per epi-chunk
            pr = psum1_pool.tile([1, NSLOT // 2], mybir.dt.float32, tag="pr")
            nc.tensor.matmul(pr[:], ones[:], rs[:], start=True, stop=True)
            total_sb = small_pool.tile([1, NSLOT // 2], mybir.dt.float32,
                                       tag="tot")
            nc.scalar.copy(total_sb[:], pr[:])
            nc.sync.dma_start(out_ext[:], total_sb[:])

    nc.compile()
    return nc


def prepare(pcs: np.ndarray, k: int):
    pcs = np.asarray(pcs, dtype=np.float32)
    in_maps, W_super, _ = build_inputs(pcs, k)
    key = (k, tuple(W_super))
    if key not in _compiled_cache:
        _compiled_cache[key] = _build_kernel(k, W_super)
    return _compiled_cache[key], in_maps


def reduce_results(results, k: int) -> np.ndarray:
    total = 0.0
    for c in range(N_CORES):
        total += results[c]["rowsums"].astype(np.float64).sum()
    return np.float32(total / (B * N * k))


def kernel(pcs: np.ndarray, k) -> np.ndarray:
    k = int(k)
    if k <= 0:
        return np.float32(np.nan)
    nc, in_maps = prepare(pcs, k)
    res = run_bass_kernel_spmd(nc, in_maps, list(range(N_CORES)))
    return reduce_results(res.results, k)
